# revision 1
# baseline (speedup 1.0000x reference)
"""CFM contrastive loss on 8 TRN2 NeuronCores.

loss = -mean(diag(log_softmax(logits))),  logits[i,j] = 2*z1_i.z2_j - |z1_i|^2 - |z2_j|^2

The |z1_i|^2 term cancels between the logsumexp and the diagonal, so with
t[i,j] = 2*z1_i.z2_j - |z2_j|^2 the loss is mean_i(log(sum_j exp(t_ij)) - t_ii).

Sharding: z1 rows are split across 8 cores (1024 rows each); every core reads
all of z2.  Per core, rowsum_i = sum_j exp(t_ij) is computed as

    sum_j exp(g_ij - C) * w_j,   g = 2*z1 @ z2^T,  w_j = exp(C - |z2_j|^2)

with C = 100 keeping both factors inside bf16/fp32 range (max g = 176, so
exp(g-C) <= e^76; terms that underflow are < 1e-9 of any row's sum).  This
splits the work cleanly across three engines with no PE prefill pass:
  - PE: one K=128 bf16 matmul per PSUM bank (g into PSUM fp32),
  - ScalarE: exp(psum - C) straight out of PSUM into a bf16 SBUF tile
    (bias is the per-instruction constant -C; no accum_out stall),
  - VectorE: tensor_tensor_reduce multiplies by w (replicated across
    partitions) and row-sum-accumulates in fp32, in 4x DVE perf mode.
The host pre-transposes/casts the operands (layout prep only), and finishes
with log + mean in float64, plus the cheap O(N*D) diagonal term.
"""

import numpy as np
import ml_dtypes

N, D = 8192, 128
NCORES = 8
SHARD = N // NCORES      # 1024 z1 rows per core
ITILES = SHARD // 128    # 8 i-tiles per core
JCHUNK = 2048            # PSUM chunk = 4 banks of 512 fp32
NCHUNKS = N // JCHUNK    # 4 chunks of j per i-tile
CSHIFT = 100.0           # range shift: exp(g - C) * exp(C - sq2)
BF16 = ml_dtypes.bfloat16

_NC_CACHE = None


def _build_nc():
    import concourse.mybir as mybir
    import concourse.tile as tile
    from concourse import bacc

    nc = bacc.Bacc(None, target_bir_lowering=False)

    z1t2 = nc.dram_tensor("z1t2", [128, SHARD], mybir.dt.bfloat16, kind="ExternalInput")
    z2t = nc.dram_tensor("z2t", [128, N], mybir.dt.bfloat16, kind="ExternalInput")
    wrep = nc.dram_tensor("wrep", [128, N], mybir.dt.bfloat16, kind="ExternalInput")
    rs = nc.dram_tensor(
        "rs", [128, ITILES * NCHUNKS], mybir.dt.float32, kind="ExternalOutput"
    )

    EXP = mybir.ActivationFunctionType.Exp

    with tile.TileContext(nc) as tc:
        with (
            tc.tile_pool(name="const", bufs=1) as cpool,
            tc.tile_pool(name="esc", bufs=2) as epool,
            tc.tile_pool(name="scr", bufs=2) as spool,
            tc.tile_pool(name="psum", bufs=2, space="PSUM") as ppool,
        ):
            z1t2_sb = cpool.tile([128, SHARD], mybir.dt.bfloat16)
            z2t_sb = cpool.tile([128, N], mybir.dt.bfloat16)
            w_sb = cpool.tile([128, N], mybir.dt.bfloat16)
            rs_parts = cpool.tile([128, ITILES * NCHUNKS], mybir.dt.float32)
            bias_sb = cpool.tile([128, 1], mybir.dt.float32)

            nc.gpsimd.memset(bias_sb[:], -CSHIFT)

            # interleave so chunk c's z2/w land just ahead of its compute
            nc.sync.dma_start(z1t2_sb[:], z1t2[:])
            for q in range(NCHUNKS):
                sl = slice(q * JCHUNK, (q + 1) * JCHUNK)
                nc.sync.dma_start(z2t_sb[:, sl], z2t[:, sl])
                nc.sync.dma_start(w_sb[:, sl], wrep[:, sl])

            for it in range(ITILES):
                lhsT = z1t2_sb[:, it * 128 : (it + 1) * 128]
                for c in range(NCHUNKS):
                    ps = ppool.tile([128, JCHUNK], mybir.dt.float32)
                    for b in range(4):
                        j0 = c * JCHUNK + b * 512
                        nc.tensor.matmul(
                            ps[:, b * 512 : (b + 1) * 512],
                            lhsT,
                            z2t_sb[:, j0 : j0 + 512],
                            start=True,
                            stop=True,
                        )
                    e_tile = epool.tile([128, JCHUNK], mybir.dt.bfloat16)
                    nc.scalar.activation(e_tile[:], ps[:], EXP, bias=bias_sb[:])
                    col = it * NCHUNKS + c
                    scr = spool.tile([128, JCHUNK], mybir.dt.bfloat16)
                    nc.vector.scalar_tensor_tensor(
                        out=scr[:],
                        in0=e_tile[:],
                        scalar=1.0,
                        in1=w_sb[:, c * JCHUNK : (c + 1) * JCHUNK],
                        op0=mybir.AluOpType.mult,
                        op1=mybir.AluOpType.mult,
                        accum_out=rs_parts[:, col : col + 1],
                    )

            # chunk partials go out raw; the host sums the NCHUNKS columns
            nc.sync.dma_start(rs[:], rs_parts[:])

    nc.compile()
    return nc


def _get_nc():
    global _NC_CACHE
    if _NC_CACHE is None:
        _NC_CACHE = _build_nc()
    return _NC_CACHE


def _prep_inputs(z1, z2):
    z1 = np.asarray(z1, dtype=np.float32)
    z2 = np.asarray(z2, dtype=np.float32)
    z2b = z2.astype(BF16)
    z2t = np.ascontiguousarray(z2b.T)  # [128, N] bf16
    sq2 = (z2b.astype(np.float64) ** 2).sum(axis=-1)  # from the bf16 values
    w = np.exp(CSHIFT - sq2).astype(np.float32).astype(BF16)
    wrep = np.ascontiguousarray(np.broadcast_to(w[None, :], (128, N)))
    in_maps = []
    for c in range(NCORES):
        z1s = z1[c * SHARD : (c + 1) * SHARD]
        z1t2 = np.ascontiguousarray((2.0 * z1s.astype(np.float64)).astype(BF16).T)
        in_maps.append({"z1t2": z1t2, "z2t": z2t, "wrep": wrep})
    return in_maps


def _finish(z1, z2, rs_list):
    # rs[p, it*NCHUNKS+c] = chunk-c partial rowsum of shard row it*128+p
    rows = np.concatenate(
        [
            np.asarray(r["rs"], np.float64)
            .reshape(128, ITILES, NCHUNKS)
            .sum(axis=2)
            .T.reshape(-1)
            for r in rs_list
        ]
    )
    z1 = np.asarray(z1, dtype=np.float64)
    z2 = np.asarray(z2, dtype=np.float64)
    sq2 = (z2.astype(BF16).astype(np.float64) ** 2).sum(axis=-1)
    tdiag = 2.0 * (z1 * z2).sum(axis=-1) - sq2
    loss = np.mean(np.log(rows) - tdiag)
    return np.asarray(loss, dtype=np.float32)


def _ensure_hook_shim():
    """bass_utils imports antenv.axon_hooks whenever tracing is requested
    (e.g. via a BASS_TRACE env var); this image's antenv lacks that module.
    Provide an inert registry so tracing degrades to a warning instead of an
    ImportError.  A previously installed real shim is left untouched."""
    import sys

    try:
        import antenv.axon_hooks  # noqa: F401
    except ImportError:
        import types

        import antenv

        mod = types.ModuleType("antenv.axon_hooks")
        mod._hook = None
        mod.set_axon_ntff_profile_hook = lambda h: setattr(mod, "_hook", h)
        mod.get_axon_ntff_profile_hook = lambda: mod._hook
        sys.modules["antenv.axon_hooks"] = mod
        antenv.axon_hooks = mod


def _run(z1, z2, **spmd_kwargs):
    _ensure_hook_shim()
    from concourse.bass_utils import run_bass_kernel_spmd

    in_maps = _prep_inputs(z1, z2)
    res = run_bass_kernel_spmd(
        _get_nc(), in_maps, core_ids=list(range(NCORES)), **spmd_kwargs
    )
    return _finish(z1, z2, res.results), res


def kernel(z1, z2):
    loss, _ = _run(z1, z2)
    return loss



# revision 3
# speedup vs baseline: 1.3729x; 1.3729x over previous
"""CFM contrastive loss on 8 TRN2 NeuronCores — transposed j-shard design.

loss = -mean(diag(log_softmax(logits))),  logits[i,j] = 2*z1_i.z2_j - |z1_i|^2 - |z2_j|^2

With t[i,j] = 2*z1_i.z2_j - |z2_j|^2 the loss is mean_i(log(sum_j exp(t_ij)) - t_ii);
the |z1_i|^2 term cancels.  t spans ~[-317, +54] but per-row only terms within
~20 of the row max matter, and row maxes span ~[-70, +54]: with a global shift
C=30, exp(t+C) fits bf16 (max e^84; terms that flush to zero are >=47 below the
weakest row max — negligible).

Sharding: z2 rows (j) are split across 8 cores (1024 each = 8 partition tiles);
every core reads all of z1 as the matmul moving operand.  Layout is transposed
vs the usual: g^T[j, i] = lhsT(z2-tile).T @ (2*z1).T, so j sits on PSUM
partitions and the per-j offset C - |z2_j|^2 rides the ACT activation's
per-partition bias — no separate w-multiply pass (the old STT ran at 1x DVE
mode, ~2.3us/tile).

Per (i-chunk, j-tile) step of [128, 2048]:
  - PE: matmul into PSUM (K=128, one 2048-wide or four 512-wide)
  - exp path, one of:
      ACT: e = exp(psum + bias_j)  -> bf16  (~1.97us/tile)
      DVE Schraudolph: i16 = rne_sat(psum*(128/ln2) + b2_j) via tensor_scalar
        (f32->i16 saturates: underflow -> -32768 = bf16 -0.0), bitcast = bf16
        2^x approx (~2.29us/tile, relerr ~1e-5 on the loss)
  - DVE: acc[:, ichunk] += e (bf16 tensor_tensor, 2x DVE mode ~1.2us/tile);
    jt==0 writes acc directly from ACT (or a 4x-mode copy for Schraudolph)
Host: partition-sum the 8 acc[128, 8192] outputs in f64, then
loss = mean(log(rows) - C - tdiag) + the cheap O(N*D) diagonal term.
"""

import numpy as np
import ml_dtypes

N, D = 8192, 128
NCORES = 8
JSHARD = N // NCORES         # 1024 z2 rows per core
JTILES = JSHARD // 128       # 8 partition tiles
ICHUNK = 2048                # PSUM chunk (4 banks of 512 fp32)
NIC = N // ICHUNK            # 4 i-chunks
CSHIFT = 30.0                # global shift: e = exp(t + C)
SCHRA_A = 128.0 / np.log(2.0)      # Schraudolph slope (bf16 bit domain)
SCHRA_B = 16256.0 - 7.0            # 127*128 + beta, beta=-7 calibrated
WIDE_MM = False              # 2048-wide matmul fails walrus ISA check: 4x512
N_SCH = 0                    # of the 32 steps, how many use DVE Schraudolph
BF16 = ml_dtypes.bfloat16

_NC_CACHE = None


def _sch_steps():
    """Which (ic, jt) steps use the DVE Schraudolph path: spread N_SCH of the
    32 steps evenly, never jt==0 (jt==0 is ACT's free direct-write)."""
    steps = [(ic, jt) for ic in range(NIC) for jt in range(JTILES)]
    cand = [s for s in steps if s[1] != 0]
    if N_SCH == 0:
        return set()
    stride = len(cand) / N_SCH
    return {cand[min(int(k * stride), len(cand) - 1)] for k in range(N_SCH)}


def _build_nc():
    import concourse.mybir as mybir
    import concourse.tile as tile
    from concourse import bacc

    nc = bacc.Bacc(None, target_bir_lowering=False)

    z1t2 = nc.dram_tensor("z1t2", [128, N], mybir.dt.bfloat16, kind="ExternalInput")
    z2t = nc.dram_tensor("z2t", [128, JSHARD], mybir.dt.bfloat16, kind="ExternalInput")
    biasd = nc.dram_tensor("biasd", [128, JTILES], mybir.dt.float32, kind="ExternalInput")
    b2d = nc.dram_tensor("b2d", [128, JTILES], mybir.dt.float32, kind="ExternalInput")
    accd = nc.dram_tensor("accd", [128, N], mybir.dt.bfloat16, kind="ExternalOutput")

    EXP = mybir.ActivationFunctionType.Exp
    ADD = mybir.AluOpType.add
    MULT = mybir.AluOpType.mult
    sch = _sch_steps()

    with tile.TileContext(nc) as tc:
        with (
            tc.tile_pool(name="const", bufs=1) as cpool,
            tc.tile_pool(name="esc", bufs=3) as epool,
            tc.tile_pool(name="psum", bufs=2, space="PSUM") as ppool,
        ):
            z1_sb = cpool.tile([128, N], mybir.dt.bfloat16)
            z2_sb = cpool.tile([128, JSHARD], mybir.dt.bfloat16)
            bias_sb = cpool.tile([128, JTILES], mybir.dt.float32)
            b2_sb = cpool.tile([128, JTILES], mybir.dt.float32)
            acc_sb = cpool.tile([128, N], mybir.dt.bfloat16)

            nc.sync.dma_start(z2_sb[:], z2t[:])
            nc.sync.dma_start(bias_sb[:], biasd[:])
            nc.sync.dma_start(b2_sb[:], b2d[:])
            for ic in range(NIC):
                sl = slice(ic * ICHUNK, (ic + 1) * ICHUNK)
                nc.sync.dma_start(z1_sb[:, sl], z1t2[:, sl])

            for ic in range(NIC):
                sl = slice(ic * ICHUNK, (ic + 1) * ICHUNK)
                for jt in range(JTILES):
                    ps = ppool.tile([128, ICHUNK], mybir.dt.float32)
                    lhsT = z2_sb[:, jt * 128 : (jt + 1) * 128]
                    if WIDE_MM:
                        nc.tensor.matmul(ps[:], lhsT, z1_sb[:, sl], start=True, stop=True)
                    else:
                        for b in range(4):
                            j0 = ic * ICHUNK + b * 512
                            nc.tensor.matmul(
                                ps[:, b * 512 : (b + 1) * 512],
                                lhsT,
                                z1_sb[:, j0 : j0 + 512],
                                start=True,
                                stop=True,
                            )
                    if (ic, jt) in sch:
                        it = epool.tile([128, ICHUNK], mybir.dt.int16)
                        nc.vector.tensor_scalar(
                            out=it[:], in0=ps[:],
                            scalar1=SCHRA_A, scalar2=b2_sb[:, jt : jt + 1],
                            op0=MULT, op1=ADD,
                        )
                        nc.vector.tensor_tensor(
                            out=acc_sb[:, sl], in0=acc_sb[:, sl],
                            in1=it[:].bitcast(mybir.dt.bfloat16), op=ADD,
                        )
                    elif jt == 0:
                        nc.scalar.activation(
                            acc_sb[:, sl], ps[:], EXP, bias=bias_sb[:, jt : jt + 1]
                        )
                    else:
                        e = epool.tile([128, ICHUNK], mybir.dt.bfloat16)
                        nc.scalar.activation(
                            e[:], ps[:], EXP, bias=bias_sb[:, jt : jt + 1]
                        )
                        nc.vector.tensor_tensor(
                            out=acc_sb[:, sl], in0=acc_sb[:, sl], in1=e[:], op=ADD
                        )
                nc.sync.dma_start(accd[:, sl], acc_sb[:, sl])

    nc.compile()
    return nc


def _get_nc():
    global _NC_CACHE
    if _NC_CACHE is None:
        _NC_CACHE = _build_nc()
    return _NC_CACHE


def _prep_inputs(z1, z2):
    z1 = np.asarray(z1, dtype=np.float32)
    z2 = np.asarray(z2, dtype=np.float32)
    z1t2 = np.ascontiguousarray((2.0 * z1.astype(np.float64)).astype(BF16).T)
    z2b = z2.astype(BF16)
    sq2 = (z2b.astype(np.float64) ** 2).sum(axis=-1)  # from the bf16 values
    bias_full = (CSHIFT - sq2).astype(np.float32)     # [N]
    b2_full = (SCHRA_A * bias_full.astype(np.float64) + SCHRA_B).astype(np.float32)
    in_maps = []
    for c in range(NCORES):
        jsl = slice(c * JSHARD, (c + 1) * JSHARD)
        z2t = np.ascontiguousarray(z2b[jsl].T)  # [128, JSHARD]
        bias = np.ascontiguousarray(
            bias_full[jsl].reshape(JTILES, 128).T  # [128, JTILES]
        )
        b2 = np.ascontiguousarray(b2_full[jsl].reshape(JTILES, 128).T)
        in_maps.append({"z1t2": z1t2, "z2t": z2t, "biasd": bias, "b2d": b2})
    return in_maps


def _finish(z1, z2, res_list):
    rows = np.zeros(N, np.float64)
    for r in res_list:
        rows += np.asarray(r["accd"], np.float64).sum(axis=0)
    z1 = np.asarray(z1, dtype=np.float64)
    z2 = np.asarray(z2, dtype=np.float64)
    sq2 = (z2.astype(BF16).astype(np.float64) ** 2).sum(axis=-1)
    tdiag = 2.0 * (z1 * z2).sum(axis=-1) - sq2
    loss = np.mean(np.log(rows) - CSHIFT - tdiag)
    return np.asarray(loss, dtype=np.float32)


def _ensure_hook_shim():
    """bass_utils imports antenv.axon_hooks whenever tracing is requested
    (e.g. via a BASS_TRACE env var); this image's antenv lacks that module.
    Provide an inert registry so tracing degrades to a warning instead of an
    ImportError.  A previously installed real shim is left untouched."""
    import sys

    try:
        import antenv.axon_hooks  # noqa: F401
    except ImportError:
        import types

        import antenv

        mod = types.ModuleType("antenv.axon_hooks")
        mod._hook = None
        mod.set_axon_ntff_profile_hook = lambda h: setattr(mod, "_hook", h)
        mod.get_axon_ntff_profile_hook = lambda: mod._hook
        sys.modules["antenv.axon_hooks"] = mod
        antenv.axon_hooks = mod


def _run(z1, z2, **spmd_kwargs):
    _ensure_hook_shim()
    from concourse.bass_utils import run_bass_kernel_spmd

    in_maps = _prep_inputs(z1, z2)
    res = run_bass_kernel_spmd(
        _get_nc(), in_maps, core_ids=list(range(NCORES)), **spmd_kwargs
    )
    return _finish(z1, z2, res.results), res


def kernel(z1, z2):
    loss, _ = _run(z1, z2)
    return loss


# revision 5
# speedup vs baseline: 1.4035x; 1.0222x over previous
"""CFM contrastive loss on 8 TRN2 NeuronCores — transposed j-shard design.

loss = -mean(diag(log_softmax(logits))),  logits[i,j] = 2*z1_i.z2_j - |z1_i|^2 - |z2_j|^2

With t[i,j] = 2*z1_i.z2_j - |z2_j|^2 the loss is mean_i(log(sum_j exp(t_ij)) - t_ii);
the |z1_i|^2 term cancels.  t spans ~[-317, +54] but per-row only terms within
~20 of the row max matter, and row maxes span ~[-70, +54]: with a global shift
C=30, exp(t+C) fits bf16 (max e^84; terms that flush to zero are >=47 below the
weakest row max — negligible).

Sharding: z2 rows (j) are split across 8 cores (1024 each = 8 partition tiles);
every core reads all of z1 as the matmul moving operand.  Layout is transposed
vs the usual: g^T[j, i] = lhsT(z2-tile).T @ (2*z1).T, so j sits on PSUM
partitions and the per-j offset C - |z2_j|^2 rides the ACT activation's
per-partition bias — no separate w-multiply pass (the old STT ran at 1x DVE
mode, ~2.3us/tile).

Per (i-chunk, j-tile) step of [128, 2048]:
  - PE: matmul into PSUM (K=128, one 2048-wide or four 512-wide)
  - exp path, one of:
      ACT: e = exp(psum + bias_j)  -> bf16  (~1.97us/tile)
      DVE Schraudolph: i16 = rne_sat(psum*(128/ln2) + b2_j) via tensor_scalar
        (f32->i16 saturates: underflow -> -32768 = bf16 -0.0), bitcast = bf16
        2^x approx (~2.29us/tile, relerr ~1e-5 on the loss)
  - DVE: acc[:, ichunk] += e (bf16 tensor_tensor, 2x DVE mode ~1.2us/tile);
    jt==0 writes acc directly from ACT (or a 4x-mode copy for Schraudolph)
Host: partition-sum the 8 acc[128, 8192] outputs in f64, then
loss = mean(log(rows) - C - tdiag) + the cheap O(N*D) diagonal term.
"""

import numpy as np
import ml_dtypes

N, D = 8192, 128
NCORES = 8
JSHARD = N // NCORES         # 1024 z2 rows per core
JTILES = JSHARD // 128       # 8 partition tiles
ICHUNK = 2048                # PSUM chunk (4 banks of 512 fp32)
NIC = N // ICHUNK            # 4 i-chunks
CSHIFT = 30.0                # global shift: e = exp(t + C)
SCHRA_A = 128.0 / np.log(2.0)      # Schraudolph slope (bf16 bit domain)
SCHRA_B = 16256.0 - 7.0            # 127*128 + beta, beta=-7 calibrated
WIDE_MM = False              # 2048-wide matmul fails walrus ISA check: 4x512
N_SCH = 7                    # of the 32 steps, how many use DVE Schraudolph
BF16 = ml_dtypes.bfloat16

_NC_CACHE = None


def _sch_steps():
    """Which (ic, jt) steps use the DVE Schraudolph path: spread N_SCH of the
    32 steps evenly, never jt==0 (jt==0 is ACT's free direct-write)."""
    steps = [(ic, jt) for ic in range(NIC) for jt in range(JTILES)]
    cand = [s for s in steps if s[1] != 0]
    if N_SCH == 0:
        return set()
    stride = len(cand) / N_SCH
    return {cand[min(int(k * stride), len(cand) - 1)] for k in range(N_SCH)}


def _build_nc():
    import concourse.mybir as mybir
    import concourse.tile as tile
    from concourse import bacc

    nc = bacc.Bacc(None, target_bir_lowering=False)

    z1t2 = nc.dram_tensor("z1t2", [128, N], mybir.dt.bfloat16, kind="ExternalInput")
    z2t = nc.dram_tensor("z2t", [128, JSHARD], mybir.dt.bfloat16, kind="ExternalInput")
    biasd = nc.dram_tensor("biasd", [128, JTILES], mybir.dt.float32, kind="ExternalInput")
    b2d = nc.dram_tensor("b2d", [128, JTILES], mybir.dt.float32, kind="ExternalInput")
    accd = nc.dram_tensor("accd", [128, N], mybir.dt.bfloat16, kind="ExternalOutput")

    EXP = mybir.ActivationFunctionType.Exp
    ADD = mybir.AluOpType.add
    MULT = mybir.AluOpType.mult
    sch = _sch_steps()

    with tile.TileContext(nc) as tc:
        with (
            tc.tile_pool(name="const", bufs=1) as cpool,
            tc.tile_pool(name="esc", bufs=3) as epool,
            tc.tile_pool(name="psum", bufs=2, space="PSUM") as ppool,
        ):
            z1_sb = cpool.tile([128, N], mybir.dt.bfloat16)
            z2_sb = cpool.tile([128, JSHARD], mybir.dt.bfloat16)
            bias_sb = cpool.tile([128, JTILES], mybir.dt.float32)
            b2_sb = cpool.tile([128, JTILES], mybir.dt.float32)
            acc_sb = cpool.tile([128, N], mybir.dt.bfloat16)

            nc.sync.dma_start(z2_sb[:], z2t[:])
            nc.sync.dma_start(bias_sb[:], biasd[:])
            nc.sync.dma_start(b2_sb[:], b2d[:])
            for ic in range(NIC):
                sl = slice(ic * ICHUNK, (ic + 1) * ICHUNK)
                nc.sync.dma_start(z1_sb[:, sl], z1t2[:, sl])

            for ic in range(NIC):
                sl = slice(ic * ICHUNK, (ic + 1) * ICHUNK)
                for jt in range(JTILES):
                    ps = ppool.tile([128, ICHUNK], mybir.dt.float32)
                    lhsT = z2_sb[:, jt * 128 : (jt + 1) * 128]
                    if WIDE_MM:
                        nc.tensor.matmul(ps[:], lhsT, z1_sb[:, sl], start=True, stop=True)
                    else:
                        for b in range(4):
                            j0 = ic * ICHUNK + b * 512
                            nc.tensor.matmul(
                                ps[:, b * 512 : (b + 1) * 512],
                                lhsT,
                                z1_sb[:, j0 : j0 + 512],
                                start=True,
                                stop=True,
                            )
                    if (ic, jt) in sch:
                        # uint16 out: f32->u16 saturates negatives to 0 =
                        # bf16 +0.0, giving exp underflow for free (i16 would
                        # leave small negatives as NaN-pattern bit garbage)
                        it = epool.tile([128, ICHUNK], mybir.dt.uint16)
                        nc.vector.tensor_scalar(
                            out=it[:], in0=ps[:],
                            scalar1=SCHRA_A, scalar2=b2_sb[:, jt : jt + 1],
                            op0=MULT, op1=ADD,
                        )
                        nc.vector.tensor_tensor(
                            out=acc_sb[:, sl], in0=acc_sb[:, sl],
                            in1=it[:].bitcast(mybir.dt.bfloat16), op=ADD,
                        )
                    elif jt == 0:
                        nc.scalar.activation(
                            acc_sb[:, sl], ps[:], EXP, bias=bias_sb[:, jt : jt + 1]
                        )
                    else:
                        e = epool.tile([128, ICHUNK], mybir.dt.bfloat16)
                        nc.scalar.activation(
                            e[:], ps[:], EXP, bias=bias_sb[:, jt : jt + 1]
                        )
                        nc.vector.tensor_tensor(
                            out=acc_sb[:, sl], in0=acc_sb[:, sl], in1=e[:], op=ADD
                        )
                nc.sync.dma_start(accd[:, sl], acc_sb[:, sl])

    nc.compile()
    return nc


def _get_nc():
    global _NC_CACHE
    if _NC_CACHE is None:
        _NC_CACHE = _build_nc()
    return _NC_CACHE


def _prep_inputs(z1, z2):
    z1 = np.asarray(z1, dtype=np.float32)
    z2 = np.asarray(z2, dtype=np.float32)
    z1t2 = np.ascontiguousarray((2.0 * z1.astype(np.float64)).astype(BF16).T)
    z2b = z2.astype(BF16)
    sq2 = (z2b.astype(np.float64) ** 2).sum(axis=-1)  # from the bf16 values
    bias_full = (CSHIFT - sq2).astype(np.float32)     # [N]
    b2_full = (SCHRA_A * bias_full.astype(np.float64) + SCHRA_B).astype(np.float32)
    in_maps = []
    for c in range(NCORES):
        jsl = slice(c * JSHARD, (c + 1) * JSHARD)
        z2t = np.ascontiguousarray(z2b[jsl].T)  # [128, JSHARD]
        bias = np.ascontiguousarray(
            bias_full[jsl].reshape(JTILES, 128).T  # [128, JTILES]
        )
        b2 = np.ascontiguousarray(b2_full[jsl].reshape(JTILES, 128).T)
        in_maps.append({"z1t2": z1t2, "z2t": z2t, "biasd": bias, "b2d": b2})
    return in_maps


def _finish(z1, z2, res_list):
    rows = np.zeros(N, np.float64)
    for r in res_list:
        rows += np.asarray(r["accd"], np.float64).sum(axis=0)
    z1 = np.asarray(z1, dtype=np.float64)
    z2 = np.asarray(z2, dtype=np.float64)
    sq2 = (z2.astype(BF16).astype(np.float64) ** 2).sum(axis=-1)
    tdiag = 2.0 * (z1 * z2).sum(axis=-1) - sq2
    loss = np.mean(np.log(rows) - CSHIFT - tdiag)
    return np.asarray(loss, dtype=np.float32)


def _ensure_hook_shim():
    """bass_utils imports antenv.axon_hooks whenever tracing is requested
    (e.g. via a BASS_TRACE env var); this image's antenv lacks that module.
    Provide an inert registry so tracing degrades to a warning instead of an
    ImportError.  A previously installed real shim is left untouched."""
    import sys

    try:
        import antenv.axon_hooks  # noqa: F401
    except ImportError:
        import types

        import antenv

        mod = types.ModuleType("antenv.axon_hooks")
        mod._hook = None
        mod.set_axon_ntff_profile_hook = lambda h: setattr(mod, "_hook", h)
        mod.get_axon_ntff_profile_hook = lambda: mod._hook
        sys.modules["antenv.axon_hooks"] = mod
        antenv.axon_hooks = mod


def _run(z1, z2, **spmd_kwargs):
    _ensure_hook_shim()
    from concourse.bass_utils import run_bass_kernel_spmd

    in_maps = _prep_inputs(z1, z2)
    res = run_bass_kernel_spmd(
        _get_nc(), in_maps, core_ids=list(range(NCORES)), **spmd_kwargs
    )
    return _finish(z1, z2, res.results), res


def kernel(z1, z2):
    loss, _ = _run(z1, z2)
    return loss
